# revision 33
# baseline (speedup 1.0000x reference)
"""Trainium2 Bass kernel for nn_Drifting (dual-softmax contrastive drift).

Computes, for x, y_pos, y_neg all [4096, 512] f32:
    dist_pos = cdist(x, y_pos); dist_neg = cdist(x, y_neg) + eye*1e6
    logit = [-dist_pos, -dist_neg] / 0.05          # [4096, 8192]
    A = sqrt(softmax_row(logit) * softmax_col(logit))
    V = (A_pos * rowsum(A_neg)) @ y_pos - (A_neg * rowsum(A_pos)) @ y_neg

Sharding: rows of x across 8 cores (512 rows each); y replicated. Per-core
layout is TRANSPOSED: scores s[j, i] with j (8192) on partitions and local
i (512) on the free dim, so column stats are free-dim reductions and the
output matmul contracts j directly.

Math (constant global rebase): with e1 = exp(-20*s + B) for one global
constant B (host-sampled so that no column over/underflows):
    A_ji = e1_ji / sqrt(R_i * C_j),   R_i = sum_j e1_ji (global row sum),
    C_j = sum_i e1_ji (global col sum, via one AllGather of per-core sums)
    U = e1 * C^-1/2;  V_i = (SU_neg*(U@y_pos) - SU_pos*(U@y_neg))_i / R_i
    R_i = sum_j sqrt(C_j) * U_ji  (fused [ones|sqrt(C)] stats matmul)
so the combine step is just C = sum of gathered sums, w2 = sqrt(C),
colexp = 1/w2 — no per-column min tracking, no ln/exp tables in combine.

Pipeline: pair-granular phase 1 (d2 PSUM [128,2,512] over 2 banks, 4-pair
pool = all 8 banks; one sqrt per pair, 2-op Pool mask), exp batches of 8
pairs with explicit dependency edges pinning the Act schedule into
[sqrt*][exp*] phases (~2 table loads per group instead of scheduler-chosen
interleaving); the last 4 pairs use per-tile exps with accum_out so the
final column sums don't bounce through the DVE queue before the last
AllGather. e1 stored BF16. Phase 2: tp/y/w2 BF16 (ybf input halves the y
DMA), 4 V matmuls + 1 stats matmul per tile; the first pos tiles' tp/y
are pre-emitted at the end of phase 1 so the PE bridges the phase
boundary. AllGather per half: 16KB of raw column sums only (measured:
collectives on this 8-core pod are latency-cheap).
"""
import numpy as np

N = 4096
D = 512
NCORES = 8
ROWS = N // NCORES          # 512 local rows (i) per core
J = 2 * N                   # 8192 concat dim
JT = J // 128               # 64 j-tiles
NPAIR = JT // 2             # 32 pair tiles
NEG0 = 32                   # first neg tile index
TEMP = 0.05
SC = -1.0 / TEMP            # -20
MASK_VAL = 1e6
GP = 8                      # pairs per act group (16 tiles)
NG = NPAIR // GP            # 4 groups
LAG = 2                     # pairs into the next group before batch emit
import os as _os
NSPLIT = int(_os.environ.get("KERNEL_NSPLIT", "2"))   # stats AllGather splits
TPS = JT // NSPLIT          # tiles per split

_CACHE = {}


def _build_nc():
    import concourse.bass as bass
    from concourse.bass import _add_dep_helper
    from concourse import bacc
    import concourse.mybir as mybir
    import concourse.tile as tile
    from concourse.masks import make_identity
    from contextlib import ExitStack

    F32 = mybir.dt.float32
    F32R = mybir.dt.float32r
    BF16 = mybir.dt.bfloat16
    Exp = mybir.ActivationFunctionType.Exp
    Sqrt = mybir.ActivationFunctionType.Sqrt
    Copy = mybir.ActivationFunctionType.Copy
    Alu = mybir.AluOpType
    AX = mybir.AxisListType.X

    nc = bacc.Bacc("TRN2", target_bir_lowering=False, debug=False,
                   num_devices=NCORES)

    ytp = nc.dram_tensor("ytp", [NPAIR, 128, 2, 5, 128], F32R,
                         kind="ExternalInput")
    ybf = nc.dram_tensor("ybf", [J, D], BF16, kind="ExternalInput")
    xm2T = nc.dram_tensor("xm2T", [128, 5, ROWS], F32R, kind="ExternalInput")
    dsel = nc.dram_tensor("dsel", [33], F32, kind="ExternalInput")
    vout = nc.dram_tensor("vout", [ROWS, D], F32, kind="ExternalOutput")
    import os
    _nocc = bool(int(os.environ.get("KERNEL_NO_CC", "0")))

    with tile.TileContext(nc) as tc, ExitStack() as top:
        st = top.enter_context(tc.tile_pool(name="st", bufs=1))
        dram = top.enter_context(tc.tile_pool(name="dram", bufs=1, space="DRAM"))

        # ---------------- static tiles ----------------
        ident = st.tile([128, 128], F32)
        make_identity(nc, ident)

        e1_all = st.tile([128, JT, ROWS], BF16)    # 64 KB/partition
        sloc_all = st.tile([128, JT], F32)         # local col sums of e1
        # per-split so each phase-2 range depends only on its own combine
        colexp = [st.tile([128, TPS], F32, name=f"cx{h}")
                  for h in range(NSPLIT)]
        w2p = [st.tile([128, 2, TPS], BF16, name=f"w2p{h}")
               for h in range(NSPLIT)]
        xm2T_sb = st.tile([128, 5, ROWS], F32R)    # -2*x^T | norms rhs chunk
        dselb = st.tile([128, 33], F32)            # mask row gains | exp bias
        stats_sb = st.tile([128, 4, 4], F32)

        # =============== phase 0: stage small inputs =======================
        nc.sync.dma_start(out=xm2T_sb[:, 0:1, :], in_=xm2T.ap()[:, 0:1, :])
        nc.sync.dma_start(out=xm2T_sb[:, 1:3, :], in_=xm2T.ap()[:, 1:3, :])
        nc.sync.dma_start(out=xm2T_sb[:, 3:5, :], in_=xm2T.ap()[:, 3:5, :])
        dsel_d = dram.tile([33], F32)
        nc.sync.dma_start(
            out=dsel_d.rearrange("(one r) -> one r", one=1),
            in_=dsel.ap().rearrange("(one r) -> one r", one=1))
        dselb_src = bass.AP(tensor=dsel_d.tensor, offset=dsel_d.offset,
                            ap=[[0, 128], [1, 33]])
        nc.sync.dma_start(out=dselb, in_=dselb_src)
        ebias = dselb[:, 32:33]

        # stats collective: one AllGather per split of the local column sums.
        HW = 128 * TPS             # words per split payload (sloc only)
        agin = [dram.tile([HW], F32, name=f"agin{h}") for h in range(NSPLIT)]
        agout = [dram.tile([NCORES, HW], F32, name=f"agout{h}",
                           addr_space="Local" if _nocc else "Shared")
                 for h in range(NSPLIT)]

        def emit_ag(h):
            ts = slice(h * TPS, (h + 1) * TPS)
            nc.sync.dma_start(
                out=agin[h][:].rearrange("(p t) -> p t", p=128),
                in_=sloc_all[:, ts])
            if _nocc:
                for r in range(NCORES):
                    nc.sync.dma_start(out=agout[h][r, :], in_=agin[h][:])
            else:
                nc.gpsimd.collective_compute(
                    "AllGather", Alu.bypass,
                    replica_groups=[list(range(NCORES))],
                    ins=[agin[h][:]], outs=[agout[h][:, :]])

        def emit_combine(h, comb):
            sg_ = comb.tile([128, NCORES, TPS], F32, tag="sg")
            nc.sync.dma_start(
                out=sg_, in_=agout[h].rearrange("r (p t) -> p r t", p=128))
            csum = comb.tile([128, TPS], F32, tag="cs")
            nc.vector.tensor_reduce(out=csum,
                                    in_=sg_.rearrange("p r t -> p t r"),
                                    op=Alu.add, axis=AX)
            w2f = comb.tile([128, TPS], F32, tag="w2")
            act_i = nc.scalar.activation(out=w2f, in_=csum, func=Sqrt)
            nc.vector.tensor_copy(out=w2p[h][:, 1, :], in_=w2f)
            nc.vector.reciprocal(out=colexp[h], in_=w2f)
            return act_i

        for h in range(NSPLIT):
            nc.vector.memset(w2p[h][:, 0, :], 1.0)

        # =============== phase 1: scores, col sums, e1 =====================
        comb_pool = top.enter_context(tc.tile_pool(name="comb", bufs=2))
        ptp = top.enter_context(tc.tile_pool(name="ptp", bufs=6))
        py4 = top.enter_context(tc.tile_pool(name="py4", bufs=3))
        pre_tp = {}
        pre_y4 = {}

        def emit_tp(t):
            h, tr = t // TPS, t % TPS
            tp_t = ptp.tile([128, ROWS], BF16, tag="tp")
            nc.vector.tensor_scalar_mul(tp_t, e1_all[:, t, :],
                                        colexp[h][:, tr:tr + 1])
            return tp_t

        def emit_y4(t):
            y4 = py4.tile([128, 4, D], BF16, tag="y4")
            nc.sync.dma_start(
                out=y4,
                in_=ybf.ap()[128 * t:128 * t + 512, :]
                    .rearrange("(a p) d -> p a d", p=128))
            return y4
        # group g's exp batch is emitted LAG pairs into group g+1 so the PE
        # has runway while the Act engine drains the batch.
        batch_at = {}
        for g in range(NG):
            p_emit = min((g + 1) * GP + LAG - 1, NPAIR - 1)
            batch_at.setdefault(p_emit, []).append(g)
        # s ring: 16 pair slots in one static tile so each group's 8 pairs
        # are CONTIGUOUS -> one [128,16,512] exp and one sloc reduce per
        # group (framework tracks RAW/WAR on the overlapping slices)
        s_ring = st.tile([128, 32, ROWS], F32)
        with tc.tile_pool(name="pyt", bufs=3) as pyt, \
             tc.tile_pool(name="pmsk", bufs=2) as pmsk, \
             tc.tile_pool(name="ps_d2", bufs=4, space="PSUM") as ps_d2:
            sqrt_insts = {}
            last_exp = None          # last exp of the most recent batch
            for P in range(NPAIR):
                if P == 0:
                    # split the first y-pair load so the c=0 chunk (the only
                    # one the first matmuls need) lands ~3x sooner
                    with tc.high_priority():
                        yt = pyt.tile([128, 2, 5, 128], F32R, tag="yt")
                        nc.sync.dma_start(out=yt[:, :, 0:1, :],
                                          in_=ytp.ap()[P][:, :, 0:1, :])
                        nc.sync.dma_start(out=yt[:, :, 1:5, :],
                                          in_=ytp.ap()[P][:, :, 1:5, :])
                else:
                    yt = pyt.tile([128, 2, 5, 128], F32R, tag="yt")
                    nc.sync.dma_start(out=yt, in_=ytp.ap()[P])
                d2 = ps_d2.tile([128, 2, ROWS], F32, tag="d2")
                for h2 in range(2):
                    for c in range(4):
                        nc.tensor.matmul(d2[:, h2, :], lhsT=yt[:, h2, c, :],
                                         rhs=xm2T_sb[:, c, :],
                                         start=(c == 0), stop=False)
                    nc.tensor.matmul(d2[:, h2, :], lhsT=yt[0:4, h2, 4, :],
                                     rhs=xm2T_sb[0:4, 4, :],
                                     start=False, stop=True)
                sl0 = 2 * (P % 16)
                sP = s_ring[:, sl0:sl0 + 2, :]
                sq_i = nc.scalar.activation(out=sP, in_=d2, func=Sqrt)
                sqrt_insts[P] = sq_i
                if last_exp is not None:
                    # keep the Act schedule in [sqrt*][exp*] phases so the
                    # static table-load pass inserts ~2 loads per group
                    _add_dep_helper(sq_i.ins, last_exp.ins,
                                    reason="act table phase order")
                if P >= NPAIR // 2:
                    for h2 in range(2):
                        m = 2 * P + h2 - NEG0
                        q4 = m % 4
                        sl = sP[:, h2, q4 * 128:(q4 + 1) * 128]
                        msk = pmsk.tile([128, 128], F32, tag="m")
                        nc.gpsimd.tensor_scalar_mul(msk, ident,
                                                    dselb[:, m:m + 1])
                        nc.gpsimd.tensor_tensor(out=sl, in0=sl, in1=msk,
                                                op=Alu.add)
                for g in batch_at.get(P, ()):
                    anchor = sqrt_insts[P]
                    # one big exp + one sloc reduce over the group's
                    # contiguous ring span; the last 4 pairs get per-tile
                    # exps with fused column-sum accumulate so the final
                    # slocs (and last AllGather) skip the DVE queue
                    r0 = 2 * ((g * GP) % 16)
                    nbig = GP - 4 if g == NG - 1 else GP
                    if nbig:
                        t0 = 2 * g * GP
                        ex_i = nc.scalar.activation(
                            out=e1_all[:, t0:t0 + 2 * nbig, :],
                            in_=s_ring[:, r0:r0 + 2 * nbig, :],
                            func=Exp, scale=SC, bias=ebias)
                        _add_dep_helper(ex_i.ins, anchor.ins,
                                        reason="act table phase order")
                        last_exp = ex_i
                        nc.vector.tensor_reduce(
                            out=sloc_all[:, t0:t0 + 2 * nbig],
                            in_=e1_all[:, t0:t0 + 2 * nbig, :],
                            op=Alu.add, axis=AX)
                    for PP in range(g * GP + nbig, (g + 1) * GP):
                        sl = 2 * (PP % 16)
                        for h2 in range(2):
                            tt = 2 * PP + h2
                            ex_i = nc.scalar.activation(
                                out=e1_all[:, tt, :],
                                in_=s_ring[:, sl + h2, :],
                                func=Exp, scale=SC, bias=ebias,
                                accum_out=sloc_all[:, tt:tt + 1])
                            _add_dep_helper(ex_i.ins, anchor.ins,
                                            reason="act table phase order")
                            last_exp = ex_i
                    GS = NG // NSPLIT
                    for q in range(NSPLIT):
                        if (q + 1) * GS - 1 == g:
                            emit_ag(q)
                    for q in range(NSPLIT):
                        if (q + 1) * GS == g:
                            # combine-q's sqrt rides the next batch boundary
                            # (sqrt table is reloaded right after these exps)
                            c_i = emit_combine(q, comb_pool)
                            _add_dep_helper(c_i.ins, last_exp.ins,
                                            reason="combine sqrt after exps")
                    if g == NG - 1:
                        # phase-2 prologue: first pos tiles' tp + y tiles are
                        # emitted here (after the AG payload DMAs) so the PE
                        # can bridge the last exp batch with V matmuls
                        pre_y4[0] = emit_y4(0)
                        pre_y4[4] = emit_y4(4)
                        for t in range(6):
                            pre_tp[t] = emit_tp(t)

        # =============== phase 2: U tiles, V and stats matmuls =============
        with tc.tile_pool(name="p2", bufs=1) as p2s, \
             tc.tile_pool(name="pvo", bufs=4) as pvo, \
             tc.tile_pool(name="ps_v", bufs=4, space="PSUM") as ps_v, \
             tc.tile_pool(name="ps_st", bufs=2, space="PSUM") as ps_st, \
             tc.tile_pool(name="ps_small", bufs=2, space="PSUM") as ps_small:
            vpos_sb = p2s.tile([128, 4, D], F32)
            spos_sb = p2s.tile([2, ROWS], F32)
            sneg_sb = p2s.tile([2, ROWS], F32)
            vps_pos = [ps_v.tile([128, D], F32, name=f"vp{b}", tag="v")
                       for b in range(4)]
            stp_pos = ps_st.tile([2, ROWS], F32, name="sp", tag="s")
            for t in range(JT):
                pos = t < NEG0
                if t == 4:
                    emit_combine(NSPLIT - 1, comb_pool)
                if t == NEG0:
                    vps_neg = [ps_v.tile([128, D], F32, name=f"vn{b}", tag="v")
                               for b in range(4)]
                    stp_neg = ps_st.tile([2, ROWS], F32, name="sn", tag="s")
                vps = vps_pos if pos else vps_neg
                stp = stp_pos if pos else stp_neg
                first, last = t in (0, NEG0), t in (NEG0 - 1, JT - 1)
                h, tr = t // TPS, t % TPS
                tp_t = pre_tp.pop(t, None) or emit_tp(t)
                if t % 4 == 0:
                    y4 = pre_y4.pop(t, None) or emit_y4(t)
                y_t = y4[:, t % 4, :]
                for b in range(4):
                    nc.tensor.matmul(vps[b][:, :],
                                     lhsT=tp_t[:, b * 128:(b + 1) * 128],
                                     rhs=y_t, start=first, stop=last)
                nc.tensor.matmul(stp[:, :], lhsT=w2p[h][:, :, tr], rhs=tp_t,
                                 start=first, stop=last)
                if t == NEG0 - 1:
                    for b in range(4):
                        nc.vector.tensor_copy(out=vpos_sb[:, b, :],
                                              in_=vps_pos[b])
                    nc.vector.tensor_copy(out=spos_sb, in_=stp_pos)
                if t == NEG0:
                    # pos-half stat transposes overlap the neg matmuls
                    for q in range(4):
                        stq = ps_small.tile([128, 2], F32, tag="sm")
                        nc.tensor.transpose(
                            stq, spos_sb[:, q * 128:(q + 1) * 128],
                            ident[0:2, 0:2])
                        nc.vector.tensor_copy(
                            out=stats_sb[:, q, 0:2], in_=stq)
            nc.vector.tensor_copy(out=sneg_sb, in_=stp_neg)

            # ---- final per-row scales and output ----
            # stats_sb[:, q, :] = [SU_pos, R_pos, SU_neg, R_neg] per i
            for q in range(4):
                stq = ps_small.tile([128, 2], F32, tag="sm")
                nc.tensor.transpose(stq, sneg_sb[:, q * 128:(q + 1) * 128],
                                    ident[0:2, 0:2])
                nc.vector.tensor_copy(out=stats_sb[:, q, 2:4], in_=stq)
            rtot = st.tile([128, 4], F32)
            nc.vector.tensor_tensor(out=rtot, in0=stats_sb[:, :, 1],
                                    in1=stats_sb[:, :, 3], op=Alu.add)
            rinv = st.tile([128, 4], F32)
            nc.vector.reciprocal(out=rinv, in_=rtot)
            sc_pos = st.tile([128, 4], F32)
            sc_negm = st.tile([128, 4], F32)       # -SU_pos/R (fused subtract)
            nc.vector.tensor_tensor(out=sc_pos, in0=stats_sb[:, :, 2],
                                    in1=rinv, op=Alu.mult)
            nc.vector.tensor_tensor(out=sc_negm, in0=stats_sb[:, :, 0],
                                    in1=rinv, op=Alu.mult)
            nc.vector.tensor_scalar_mul(sc_negm, sc_negm, -1.0)
            for b in range(4):
                t1 = pvo.tile([128, D], F32, tag="t1")
                nc.scalar.activation(out=t1, in_=vpos_sb[:, b, :], func=Copy,
                                     scale=sc_pos[:, b:b + 1])
                vo = pvo.tile([128, D], F32, tag="vo")
                nc.vector.scalar_tensor_tensor(
                    out=vo, in0=vps_neg[b], scalar=sc_negm[:, b:b + 1],
                    in1=t1, op0=Alu.mult, op1=Alu.add)
                nc.sync.dma_start(out=vout.ap()[b * 128:(b + 1) * 128, :],
                                  in_=vo)
    nc.finalize()
    return nc


def _f32r_np(a):
    """Round to f32r (tf32-like, 10 explicit mantissa bits) on host."""
    a = np.asarray(a, np.float32)
    u = a.view(np.uint32)
    u2 = (u + np.uint32(1 << 12)) & np.uint32((0xFFFFFFFF << 13) & 0xFFFFFFFF)
    return u2.view(np.float32)


def _in_maps(x, y_pos, y_neg):
    import ml_dtypes
    x = np.asarray(x, np.float32)
    y_pos = np.asarray(y_pos, np.float32)
    y_neg = np.asarray(y_neg, np.float32)
    ycat = np.concatenate([y_pos, y_neg], axis=0)          # (J, D)
    yn = np.sum(ycat.astype(np.float64) * ycat, axis=1).astype(np.float32)
    # global rebase bias B = 20*s0 + 45 with s0 = sampled min distance:
    # keeps every column's max e1 in f32/bf16 normal range (col-min spread
    # is ~4.8 in s units) while e1max <= ~e^60.
    sub = x[::16]                                          # 256 sample rows
    d2s = (np.sum(sub * sub, axis=1)[None, :] + yn[:, None]
           - 2.0 * (ycat @ sub.T))
    s0 = float(np.sqrt(max(float(d2s.min()), 0.0)))
    bias1 = 20.0 * s0 + 45.0
    # y^T tiles (chunks 0-3): base[t, d, c, j] = ycat[t*128 + j, c*128 + d].
    # Chunk 4 (K=4) adds the norms: d2 += 1*xn_hi + 1*xn_lo + yn_hi*1 + yn_lo*1
    base = np.zeros((JT, 128, 5, 128), np.float32)
    base[:, :, 0:4, :] = ycat.reshape(JT, 128, 4, 128).transpose(0, 3, 2, 1)
    yn_hi = _f32r_np(yn)
    yn_lo = _f32r_np(yn - yn_hi)
    base[:, 0, 4, :] = 1.0
    base[:, 1, 4, :] = 1.0
    base[:, 2, 4, :] = yn_hi.reshape(JT, 128)
    base[:, 3, 4, :] = yn_lo.reshape(JT, 128)
    # pair layout: ytp[P, d, h2, c, j] = base[2P + h2, d, c, j]
    ytp = np.ascontiguousarray(
        base.reshape(NPAIR, 2, 128, 5, 128).transpose(0, 2, 1, 3, 4))
    ybf = np.asarray(ycat, ml_dtypes.bfloat16)
    maps = []
    for k in range(NCORES):
        xs = x[k * ROWS:(k + 1) * ROWS]                    # (ROWS, D)
        xm2T = np.zeros((128, 5, ROWS), np.float32)
        xm2T[:, 0:4, :] = (-2.0 * xs.T).reshape(4, 128, ROWS).transpose(1, 0, 2)
        xn = np.sum(xs.astype(np.float64) * xs, axis=1).astype(np.float32)
        xm2T[0, 4, :] = _f32r_np(xn)
        xm2T[1, 4, :] = _f32r_np(xn - xm2T[0, 4, :])
        xm2T[2, 4, :] = 1.0
        xm2T[3, 4, :] = 1.0
        ds = np.zeros(33, np.float32)
        ds[4 * k:4 * k + 4] = MASK_VAL
        ds[32] = bias1
        maps.append({
            "ytp": ytp,
            "ybf": ybf,
            "xm2T": xm2T,
            "dsel": ds,
        })
    return maps


def _get_runner():
    """Build (once) the jitted 8-core shard_map executable, mirroring
    concourse.bass2jax.run_bass_via_pjrt. Returns a dict with the jit, input
    name order, zero-output templates and output names."""
    if "runner" in _CACHE:
        return _CACHE["runner"]
    import jax
    import jax.numpy as jnp
    from jax.sharding import Mesh, PartitionSpec
    from jax.experimental.shard_map import shard_map
    import concourse.mybir as mybir
    from concourse.bass2jax import (_bass_exec_p, install_neuronx_cc_hook,
                                    partition_id_tensor)

    install_neuronx_cc_hook()
    nc = _build_nc()

    partition_name = (nc.partition_id_tensor.name
                      if nc.partition_id_tensor else None)
    in_names, out_names, out_avals, zero_outs = [], [], [], []
    for alloc in nc.m.functions[0].allocations:
        if not isinstance(alloc, mybir.MemoryLocationSet):
            continue
        if not alloc.memorylocations:
            continue
        name = alloc.memorylocations[0].name
        if alloc.kind == "ExternalInput":
            if name != partition_name:
                in_names.append(name)
        elif alloc.kind == "ExternalOutput":
            shape = tuple(alloc.tensor_shape)
            dtype = mybir.dt.np(alloc.dtype)
            out_names.append(name)
            out_avals.append(jax.core.ShapedArray(shape, dtype))
            zero_outs.append(np.zeros(shape, dtype))
    n_params = len(in_names)
    n_outs = len(out_avals)
    all_in_names = in_names + out_names
    if partition_name is not None:
        all_in_names = all_in_names + [partition_name]
    donate = tuple(range(n_params, n_params + n_outs))

    def _body(*args):
        operands = list(args)
        if partition_name is not None:
            operands.append(partition_id_tensor())
        outs = _bass_exec_p.bind(
            *operands,
            out_avals=tuple(out_avals),
            in_names=tuple(all_in_names),
            out_names=tuple(out_names),
            lowering_input_output_aliases=(),
            sim_require_finite=True,
            sim_require_nnan=True,
            nc=nc,
        )
        return tuple(outs)

    devices = jax.devices()[:NCORES]
    mesh = Mesh(np.asarray(devices), ("core",))
    in_specs = (PartitionSpec("core"),) * (n_params + n_outs)
    out_specs = (PartitionSpec("core"),) * n_outs
    sharded = jax.jit(
        shard_map(_body, mesh=mesh, in_specs=in_specs, out_specs=out_specs,
                  check_rep=False),
        donate_argnums=donate, keep_unused=True)

    runner = dict(sharded=sharded, in_names=in_names, out_names=out_names,
                  out_avals=out_avals, zero_outs=zero_outs, mesh=mesh,
                  n_params=n_params)
    _CACHE["runner"] = runner
    return runner


def _concat_inputs(runner, in_maps):
    return [np.concatenate([np.asarray(in_maps[c][name])
                            for c in range(NCORES)], axis=0)
            for name in runner["in_names"]]


def _concat_zeros(runner):
    return [np.zeros((NCORES * z.shape[0], *z.shape[1:]), z.dtype)
            for z in runner["zero_outs"]]


def _run(in_maps):
    runner = _get_runner()
    out_arrs = runner["sharded"](*_concat_inputs(runner, in_maps),
                                 *_concat_zeros(runner))
    outs = {}
    for i, name in enumerate(runner["out_names"]):
        shp = runner["out_avals"][i].shape
        outs[name] = np.asarray(out_arrs[i]).reshape(NCORES, *shp)
    return outs


def kernel(x, y_pos, y_neg):
    in_dtype = np.asarray(x).dtype
    outs = _run(_in_maps(x, y_pos, y_neg))
    v = outs["vout"].reshape(N, D)
    return np.ascontiguousarray(v).astype(in_dtype, copy=False)
